# revision 1
# baseline (speedup 1.0000x reference)
"""2-layer GCN (PyG-style GCNConv) on 8 Trainium2 NeuronCores.

Strategy
--------
out = A_hat @ (A_hat @ x W1 + b1).relu() @ W2 + b2   with
A_hat = D^-1/2 (A + I) D^-1/2.  Since A_hat is linear we aggregate FIRST
(A_hat x) and transform after, so both layers gather raw feature rows.

* Nodes (padded to 50176) are sharded 6272/core; each core owns its dst rows.
* Edges are grouped by (dst-block of 128, src-half) and chunked into 128-edge
  chunks.  Per chunk the host builds a [128 src-slot, 128 dst] bf16 selection
  matrix holding norm = dinv[src]*dinv[dst] (zero for pad slots), so the PE
  does gather-side scale + segment-sum as one matmul chain into PSUM.
* Slot source rows are fetched with gpsimd dma_gather (int16 indices), which
  is per-index bound, so layer 1 gathers 512B x-rows at the same cost as
  layer 2's 256B h-rows.  Two gather tables per layer (src-half A/B) keep
  indices < 32768.
* Self-loops never hit the gather path: a diagonal matmul per dst block adds
  dinv[d]^2 * row_d from the core-local shard.
* Between layers the cores AllGather h2 (the relu'd layer-1 output) in two
  halves so the layer-2 A-phase gathers overlap the second collective.
"""

import os
import sys

sys.path.insert(0, "/opt/trn_rl_repo")

import numpy as np
import ml_dtypes

import concourse.bacc as bacc
import concourse.bass as bass
import concourse.mybir as mybir
from concourse.bass_utils import run_bass_kernel_spmd
from concourse.tile import TileContext
from concourse.library_config import mlp

BF16 = mybir.dt.bfloat16
FP32 = mybir.dt.float32
I16 = mybir.dt.int16
NPBF16 = ml_dtypes.bfloat16

N_CORES = 8
N_RAW = 50000
SHARD = 6272                      # nodes per core (50176 total, padded)
N_PAD = SHARD * N_CORES
NBLK = SHARD // 128               # 49 dst blocks per core
HALF_A = 3200                     # shard rows [0, 3200) -> table A (25 blocks)
HALF_B = SHARD - HALF_A           # shard rows [3200, 6272) -> table B (24 blocks)
NBLK_A = HALF_A // 128
IN_CH = 256
HID = 128
OUT_CH = 128
CALL_CHUNKS = 8                   # chunks (128 idxs each) per dma_gather call

last_exec_time_ns = None
last_results = None


# ---------------------------------------------------------------- host prep

def _prep(x, edge_index):
    src = np.asarray(edge_index[0], dtype=np.int64)
    dst = np.asarray(edge_index[1], dtype=np.int64)

    deg = np.bincount(dst, minlength=N_PAD).astype(np.float64) + 1.0
    dinv = 1.0 / np.sqrt(deg)
    norm = (dinv[src] * dinv[dst]).astype(np.float32)

    core = dst // SHARD
    blk = (dst % SHARD) // 128
    soff = src % SHARD
    half = (soff >= HALF_A).astype(np.int64)          # 0 = A, 1 = B
    srank = src // SHARD
    # gather-table rows are stored K-major within each rank block:
    # row(srank, soff) = srank*HALF + (soff%128)*NBLK_half + soff//128
    boffB = soff - HALF_A
    tbl_idx = np.where(
        half == 0,
        srank * HALF_A + (soff % 128) * NBLK_A + soff // 128,
        srank * HALF_B + (boffB % 128) * (NBLK - NBLK_A) + boffB // 128,
    ).astype(np.int16)
    dst_off = (dst % 128).astype(np.int64)

    # group id: core * (2*NBLK) + half*NBLK + blk ; count per group
    gid = core * (2 * NBLK) + half * NBLK + blk
    counts = np.bincount(gid, minlength=N_CORES * 2 * NBLK).reshape(N_CORES, 2, NBLK)
    kA = np.maximum(1, np.ceil(counts[:, 0, :].max(axis=0) / 128).astype(np.int64))  # [NBLK]
    kB = np.maximum(1, np.ceil(counts[:, 1, :].max(axis=0) / 128).astype(np.int64))
    CA, CB = int(kA.sum()), int(kB.sum())
    C = CA + CB
    baseA = np.concatenate([[0], np.cumsum(kA)])[:-1]            # chunk base per block
    baseB = CA + np.concatenate([[0], np.cumsum(kB)])[:-1]

    # slot base per (half, blk)
    slot_base = np.empty((2, NBLK), dtype=np.int64)
    slot_base[0] = baseA * 128
    slot_base[1] = baseB * 128

    # position of each edge within its (core, half, blk) group
    order = np.lexsort((dst, half, blk, core))
    pos = np.empty_like(order)
    gsort = gid[order]
    first = np.concatenate([[True], gsort[1:] != gsort[:-1]])
    grp_start = np.flatnonzero(first)
    within = np.arange(order.size) - np.repeat(grp_start, np.diff(np.concatenate([grp_start, [order.size]])))
    pos[order] = within

    slot = slot_base[half, blk] + pos                             # per-edge slot (core-local)

    S = C * 128
    # sel stored K-major: sel[core, k, c, m] with slot = c*128 + k
    sel = np.zeros((N_CORES, 128, C, 128), dtype=NPBF16)
    idx16 = np.zeros((N_CORES, S), dtype=np.int16)
    flat = core * (128 * C * 128) + (slot % 128) * (C * 128) + (slot // 128) * 128 + dst_off
    sel.reshape(-1)[flat] = norm.astype(NPBF16)
    idx16.reshape(-1)[core * S + slot] = tbl_idx

    # wrap idxs: slot j -> partition j%16, col j//16; replicate to 128 partitions
    idx_w = idx16.reshape(N_CORES, S // 16, 16).transpose(0, 2, 1)  # [cores, 16, S/16]
    idx_w = np.ascontiguousarray(idx_w)
    idx_w = np.tile(idx_w, (1, 8, 1))                               # [cores, 128, S/16]

    # diag stored K-major: diag[core, k, b, m] = (k==m) * dinv^2[node b*128+k]
    dinv32 = dinv.astype(np.float32)
    diag = np.zeros((N_CORES, 128, NBLK, 128), dtype=NPBF16)
    ar = np.arange(128)
    for c in range(N_CORES):
        d2 = (dinv32[c * SHARD:(c + 1) * SHARD] ** 2).reshape(NBLK, 128)
        diag[c, ar[:, None], np.arange(NBLK)[None, :], ar[:, None]] = d2.T.astype(NPBF16)

    # x tables, K-major within each rank half: row = rank*HALF + k*NB + b
    xp = np.zeros((N_PAD, IN_CH), dtype=NPBF16)
    xp[:N_RAW] = x.astype(NPBF16)
    xr = xp.reshape(N_CORES, SHARD, IN_CH)
    NB_B = NBLK - NBLK_A
    xA = np.ascontiguousarray(
        xr[:, :HALF_A].reshape(N_CORES, NBLK_A, 128, IN_CH)
        .transpose(0, 2, 1, 3).reshape(N_CORES * HALF_A, IN_CH))
    xB = np.ascontiguousarray(
        xr[:, HALF_A:].reshape(N_CORES, NB_B, 128, IN_CH)
        .transpose(0, 2, 1, 3).reshape(N_CORES * HALF_B, IN_CH))
    # xs K-major: [128, NBLK, IN_CH]
    xs = np.ascontiguousarray(
        xr.reshape(N_CORES, NBLK, 128, IN_CH).transpose(0, 2, 1, 3))

    return dict(kA=kA, kB=kB, CA=CA, CB=CB, sel=sel, idx_w=idx_w, diag=diag,
                xA=xA, xB=xB, xs=xs)


def _calls(nchunks, base):
    """Split [base, base+nchunks) chunk range into gather calls."""
    out = []
    c = 0
    while c < nchunks:
        n = min(CALL_CHUNKS, nchunks - c)
        out.append((base + c, n))
        c += n
    return out


# ----------------------------------------------------------- device program

def _build(kA, kB, CA, CB):
    C = CA + CB
    S = C * 128
    nc = bacc.Bacc("TRN2", target_bir_lowering=False, num_devices=N_CORES,
                   num_swdge_queues=4)

    xA_d = nc.dram_tensor("xA", [N_CORES * HALF_A, IN_CH], BF16, kind="ExternalInput")
    xB_d = nc.dram_tensor("xB", [N_CORES * HALF_B, IN_CH], BF16, kind="ExternalInput")
    xs_d = nc.dram_tensor("xs", [128, NBLK * IN_CH], BF16, kind="ExternalInput")
    sel_d = nc.dram_tensor("sel", [128, C * 128], BF16, kind="ExternalInput")
    idx_d = nc.dram_tensor("idx", [128, S // 16], I16, kind="ExternalInput")
    diag_d = nc.dram_tensor("diag", [128, NBLK * 128], BF16, kind="ExternalInput")
    w1_d = nc.dram_tensor("w1", [IN_CH, HID], BF16, kind="ExternalInput")
    w2_d = nc.dram_tensor("w2", [HID, OUT_CH], BF16, kind="ExternalInput")
    b1_d = nc.dram_tensor("b1", [1, HID], BF16, kind="ExternalInput")
    b2_d = nc.dram_tensor("b2", [1, OUT_CH], BF16, kind="ExternalInput")
    ident_d = nc.dram_tensor("ident", [128, 128], BF16, kind="ExternalInput")
    ones_d = nc.dram_tensor("ones", [1, 128], BF16, kind="ExternalInput")
    y_d = nc.dram_tensor("y", [SHARD, OUT_CH], FP32, kind="ExternalOutput")

    bncA = nc.dram_tensor("bncA", [128, NBLK_A * HID], BF16)
    bncB = nc.dram_tensor("bncB", [128, (NBLK - NBLK_A) * HID], BF16)
    tA = nc.dram_tensor("tA", [N_CORES * HALF_A, HID], BF16, addr_space="Shared")
    tB = nc.dram_tensor("tB", [N_CORES * HALF_B, HID], BF16, addr_space="Shared")

    RG = [list(range(N_CORES))]
    kAl, kBl = [int(v) for v in kA], [int(v) for v in kB]
    baseA = np.concatenate([[0], np.cumsum(kAl)])[:-1]
    baseB = CA + np.concatenate([[0], np.cumsum(kBl)])[:-1]

    with TileContext(nc) as tc:
        nc.gpsimd.load_library(mlp)
        import contextlib
        st = contextlib.ExitStack()
        with st:
            consts = st.enter_context(tc.tile_pool(name="consts", bufs=1))
            gpool = st.enter_context(tc.tile_pool(name="gpool", bufs=14))
            spool = st.enter_context(tc.tile_pool(name="spool", bufs=12))
            apool = st.enter_context(tc.tile_pool(name="apool", bufs=49))
            fpool = st.enter_context(tc.tile_pool(name="fpool", bufs=4))
            opool = st.enter_context(tc.tile_pool(name="opool", bufs=4))
            aggps = st.enter_context(tc.tile_pool(name="aggps", bufs=5, space="PSUM"))
            tps = st.enter_context(tc.tile_pool(name="tps", bufs=2, space="PSUM"))
            mmps = st.enter_context(tc.tile_pool(name="mmps", bufs=1, space="PSUM"))

            # ---- constants
            idx_t = consts.tile([128, S // 16], I16)
            nc.sync.dma_start(out=idx_t[:], in_=idx_d[:])
            diag_t = consts.tile([128, NBLK, 128], BF16)
            nc.sync.dma_start(out=diag_t[:], in_=diag_d[:])
            w1_t = consts.tile([128, 2, HID], BF16)
            nc.sync.dma_start(out=w1_t[:], in_=w1_d.rearrange("(c k) m -> k c m", k=128))
            w2_t = consts.tile([128, OUT_CH], BF16)
            nc.sync.dma_start(out=w2_t[:], in_=w2_d[:])
            b1_t = consts.tile([1, HID], BF16)
            nc.sync.dma_start(out=b1_t[:], in_=b1_d[:])
            b2_t = consts.tile([1, OUT_CH], BF16)
            nc.sync.dma_start(out=b2_t[:], in_=b2_d[:])
            ones_t = consts.tile([1, 128], BF16)
            nc.sync.dma_start(out=ones_t[:], in_=ones_d[:])
            ident_t = consts.tile([128, 128], BF16)
            nc.sync.dma_start(out=ident_t[:], in_=ident_d[:])
            xs_t = consts.tile([128, NBLK, IN_CH], BF16)
            nc.sync.dma_start(out=xs_t[:], in_=xs_d[:])
            h2_t = consts.tile([128, NBLK, HID], BF16)

            def agg_layer(layer, tblA, tblB, tbl_ch, rhs_diag, Wt, nW, bias_t, out_cb):
                """One GCN layer: gather+select+segsum then transform."""
                aggA = {}
                for phase, (tbl, base_list, k_list, pbase, pcnt) in enumerate(
                        [(tblA, baseA, kAl, 0, CA), (tblB, baseB, kBl, CA, CB)]):
                    gt = {}
                    for ci, (c0, n) in enumerate(_calls(pcnt, pbase)):
                        g = gpool.tile([128, CALL_CHUNKS, tbl_ch], BF16, tag="g",
                                       name=f"g{layer}_{phase}_{c0}")
                        nc.gpsimd.dma_gather(
                            g[:, :n, :], tbl[:], idx_t[:, c0 * 8:(c0 + n) * 8],
                            n * 128, n * 128, tbl_ch, queue_num=ci % 4)
                        s = spool.tile([128, CALL_CHUNKS, 128], BF16, tag="s",
                                       name=f"s{layer}_{phase}_{c0}")
                        nc.scalar.dma_start(
                            out=s[:, :n, :],
                            in_=sel_d[:, c0 * 128:(c0 + n) * 128])
                        for i in range(n):
                            gt[c0 + i] = (g, s, i)
                    for b in range(NBLK):
                        ps = aggps.tile([128, IN_CH], FP32, tag="aggps", name=f"ps{layer}_{phase}_{b}")
                        if phase == 0:
                            nc.tensor.matmul(ps[:, :tbl_ch], diag_t[:, b, :],
                                             rhs_diag(b), start=True, stop=False)
                        else:
                            nc.tensor.matmul(ps[:, :tbl_ch], ident_t[:],
                                             aggA[b][:, :tbl_ch], start=True, stop=False)
                        k_n = k_list[b]
                        for j in range(k_n):
                            cg = int(base_list[b]) + j
                            g, s, i = gt[cg]
                            nc.tensor.matmul(ps[:, :tbl_ch], s[:, i, :],
                                             g[:, i, :],
                                             start=False, stop=(j == k_n - 1))
                        if phase == 0:
                            a = apool.tile([128, IN_CH], BF16, tag="aggA", name=f"aggA{layer}_{b}")
                            aggA[b] = a
                            nc.scalar.activation(a[:, :tbl_ch], ps[:, :tbl_ch],
                                                 mybir.ActivationFunctionType.Copy)
                        else:
                            f = fpool.tile([128, IN_CH], BF16, tag="aggF", name=f"aggF{layer}_{b}")
                            nc.scalar.activation(f[:, :tbl_ch], ps[:, :tbl_ch],
                                                 mybir.ActivationFunctionType.Copy)
                            # transform: transpose chunks, matmul with W, bias, evict
                            mp = mmps.tile([128, 128], FP32, tag="mmps", name=f"mm{layer}_{b}")
                            for kc in range(nW):
                                tp = tps.tile([128, 128], BF16, tag="tp", name=f"tp{layer}_{b}_{kc}")
                                nc.tensor.transpose(tp[:], f[:, kc * 128:(kc + 1) * 128],
                                                    ident_t[:])
                                ft = fpool.tile([128, 128], BF16, tag="fT", name=f"fT{layer}_{b}_{kc}")
                                nc.scalar.activation(ft[:], tp[:],
                                                     mybir.ActivationFunctionType.Copy)
                                nc.tensor.matmul(mp[:], ft[:],
                                                 Wt(kc), start=(kc == 0), stop=False)
                            nc.tensor.matmul(mp[:], ones_t[:], bias_t[:],
                                             start=False, stop=True)
                            out_cb(b, mp)

            # ---------------- layer 1 ----------------
            def l1_out(b, mp):
                nc.scalar.activation(h2_t[:, b, :], mp[:],
                                     mybir.ActivationFunctionType.Relu)
                if b == NBLK_A - 1:
                    nc.sync.dma_start(out=bncA[:], in_=h2_t[:, :NBLK_A, :])
                    nc.gpsimd.collective_compute(
                        "AllGather", mybir.AluOpType.bypass, replica_groups=RG,
                        ins=[bncA[:]], outs=[tA[:]])
                elif b == NBLK - 1:
                    nc.sync.dma_start(out=bncB[:], in_=h2_t[:, NBLK_A:, :])
                    nc.gpsimd.collective_compute(
                        "AllGather", mybir.AluOpType.bypass, replica_groups=RG,
                        ins=[bncB[:]], outs=[tB[:]])

            agg_layer(1, xA_d, xB_d, IN_CH,
                      rhs_diag=lambda b: xs_t[:, b, :],
                      Wt=lambda kc: w1_t[:, kc, :], nW=2, bias_t=b1_t, out_cb=l1_out)

            # ---------------- layer 2 ----------------
            def l2_out(b, mp):
                o = opool.tile([128, OUT_CH], FP32, tag="o", name=f"y{b}")
                nc.scalar.activation(o[:], mp[:], mybir.ActivationFunctionType.Copy)
                nc.sync.dma_start(out=y_d[b * 128:(b + 1) * 128, :], in_=o[:])

            agg_layer(2, tA, tB, HID,
                      rhs_diag=lambda b: h2_t[:, b, :],
                      Wt=lambda kc: w2_t[:], nW=1, bias_t=b2_t, out_cb=l2_out)

    nc.compile()
    return nc


# ------------------------------------------------------------------- kernel

def kernel(x, edge_index, W1, b1, W2, b2):
    global last_exec_time_ns, last_results
    x = np.asarray(x)
    prep = _prep(np.asarray(x, dtype=np.float32), np.asarray(edge_index))
    nc = _build(prep["kA"], prep["kB"], prep["CA"], prep["CB"])

    w1b = np.asarray(W1, dtype=np.float32).astype(NPBF16)
    w2b = np.asarray(W2, dtype=np.float32).astype(NPBF16)
    b1b = np.asarray(b1, dtype=np.float32).reshape(1, -1).astype(NPBF16)
    b2b = np.asarray(b2, dtype=np.float32).reshape(1, -1).astype(NPBF16)
    ident = np.zeros((128, 128), dtype=NPBF16)
    ident[np.arange(128), np.arange(128)] = 1.0

    in_maps = []
    for c in range(N_CORES):
        in_maps.append({
            "xA": prep["xA"], "xB": prep["xB"],
            "xs": prep["xs"][c].reshape(128, -1),
            "sel": prep["sel"][c].reshape(128, -1), "idx": prep["idx_w"][c],
            "diag": prep["diag"][c].reshape(128, -1),
            "w1": w1b, "w2": w2b, "b1": b1b, "b2": b2b, "ident": ident,
            "ones": np.ones((1, 128), dtype=NPBF16),
        })

    trace = bool(int(os.environ.get("GCN_TRACE", "0")))
    if trace:
        try:
            import ntff_shim
            ntff_shim.install()
        except Exception:
            trace = False
    res = run_bass_kernel_spmd(nc, in_maps, list(range(N_CORES)), trace=trace)
    last_exec_time_ns = res.exec_time_ns
    last_results = res

    y = np.concatenate([np.asarray(res.results[c]["y"]) for c in range(N_CORES)], axis=0)
    return np.ascontiguousarray(y[:N_RAW]).astype(np.float32)



# revision 10
# speedup vs baseline: 1.1118x; 1.1118x over previous
"""2-layer GCN (PyG-style GCNConv) on 8 Trainium2 NeuronCores.

Strategy (v2)
-------------
out = A_hat @ relu(A_hat @ x W1 + b1) @ W2 + b2,  A_hat = D^-1/2 (A+I) D^-1/2.
Aggregate first (A_hat is linear), transform after.

* dinv folding: gather-table rows are pre-scaled by dinv[src]; the aggregated
  PSUM is post-scaled by dinv[dst] at eviction.  Self-loops become ordinary
  edges.  The per-chunk selection matrix is then a pure {0,1} one-hot and is
  stored RESIDENT in SBUF as fp8e4 (exact), loaded once - instead of
  streaming 51MB of bf16 norm matrices from HBM.
* Nodes (padded to 50176) sharded 6272/core; edges partitioned by dst core,
  grouped by (dst-block-of-128, src-table-half) into 128-edge chunks.
  Per chunk one matmul (one-hot lhsT, gathered rows rhs) does scatter+sum.
* Gathers use gpsimd dma_gather with prepare_only+trigger_dma so descriptor
  generation pipelines with the SDMA transfers; calls cover GROUP_BLKS dst
  blocks on rotating SWDGE queues.
* Layer 1 interleaves A/B table chunks in one PSUM accumulation chain.
  Layer 2 runs phase A (table tA, available right after the first
  half-AllGather of h) across all blocks, stashing partial sums in SBUF,
  then phase B once tB lands - overlapping gather work with the collective.
"""

import os
import sys

sys.path.insert(0, "/opt/trn_rl_repo")

import numpy as np
import ml_dtypes

import concourse.bacc as bacc
import concourse.bass as bass
import concourse.mybir as mybir
from concourse.bass_utils import run_bass_kernel_spmd
from concourse.tile import TileContext
from concourse.library_config import mlp

BF16 = mybir.dt.bfloat16
FP32 = mybir.dt.float32
FP8 = mybir.dt.float8e4
I16 = mybir.dt.int16
NPBF16 = ml_dtypes.bfloat16
NPFP8 = ml_dtypes.float8_e4m3

N_CORES = 8
N_RAW = 50000
SHARD = 6272                      # nodes per core (50176 total, padded)
N_PAD = SHARD * N_CORES
NBLK = SHARD // 128               # 49 dst blocks per core
HALF_A = 3200                     # shard rows [0, 3200) -> table A
HALF_B = SHARD - HALF_A           # shard rows [3200, 6272) -> table B
NBLK_A = HALF_A // 128            # 25
NBLK_B = NBLK - NBLK_A            # 24
IN_CH = 256
HID = 128
OUT_CH = 128
GROUP_BLKS = 1                    # dst blocks per gather call

last_exec_time_ns = None
last_results = None


# ---------------------------------------------------------------- host prep

def _prep(x, edge_index):
    src = np.asarray(edge_index[0], dtype=np.int64)
    dst = np.asarray(edge_index[1], dtype=np.int64)

    deg = np.bincount(dst, minlength=N_PAD).astype(np.float64) + 1.0
    dinv64 = 1.0 / np.sqrt(deg)
    dinv = dinv64.astype(np.float32)

    # self-loops as ordinary edges (for every padded node)
    loop = np.arange(N_PAD, dtype=np.int64)
    src = np.concatenate([src, loop])
    dst = np.concatenate([dst, loop])

    core = dst // SHARD
    blk = (dst % SHARD) // 128
    soff = src % SHARD
    half = (soff >= HALF_A).astype(np.int64)          # 0 = A, 1 = B
    srank = src // SHARD
    boffB = soff - HALF_A
    tbl_idx = np.where(
        half == 0,
        srank * HALF_A + (soff % 128) * NBLK_A + soff // 128,
        srank * HALF_B + (boffB % 128) * NBLK_B + boffB // 128,
    ).astype(np.int16)
    dst_off = (dst % 128).astype(np.int64)

    # chunk counts per (block, half): max over cores (SPMD shared layout)
    gid = core * (2 * NBLK) + half * NBLK + blk
    counts = np.bincount(gid, minlength=N_CORES * 2 * NBLK).reshape(N_CORES, 2, NBLK)
    kA = np.maximum(1, np.ceil(counts[:, 0, :].max(axis=0) / 128).astype(np.int64))
    kB = np.maximum(1, np.ceil(counts[:, 1, :].max(axis=0) / 128).astype(np.int64))

    # global chunk layout: per group g of GROUP_BLKS blocks:
    #   [A-chunks of blocks in g][B-chunks of blocks in g]
    groups = [list(range(g, min(g + GROUP_BLKS, NBLK)))
              for g in range(0, NBLK, GROUP_BLKS)]
    baseA = np.zeros(NBLK, dtype=np.int64)
    baseB = np.zeros(NBLK, dtype=np.int64)
    gArange = []                                       # (chunk0, nchunks) per group
    gBrange = []
    c = 0
    for blks in groups:
        a0 = c
        for b in blks:
            baseA[b] = c
            c += int(kA[b])
        gArange.append((a0, c - a0))
        b0 = c
        for b in blks:
            baseB[b] = c
            c += int(kB[b])
        gBrange.append((b0, c - b0))
    C = c                                              # total chunks per layer
    S = C * 128

    # per-edge slot
    cb = np.where(half == 0, baseA[blk], baseB[blk])
    order = np.lexsort((dst, half, blk, core))
    gsort = gid[order]
    first = np.concatenate([[True], gsort[1:] != gsort[:-1]])
    grp_start = np.flatnonzero(first)
    within = np.arange(order.size) - np.repeat(
        grp_start, np.diff(np.concatenate([grp_start, [order.size]])))
    pos = np.empty_like(order)
    pos[order] = within
    slot = cb * 128 + pos                              # core-local slot id

    # sel: fp8 one-hot, stored K-major: sel[core, k, c, m], slot = c*128 + k
    sel = np.zeros((N_CORES, 128, C, 128), dtype=NPFP8)
    flat = core * (128 * C * 128) + (slot % 128) * (C * 128) + (slot // 128) * 128 + dst_off
    sel.reshape(-1)[flat] = NPFP8(1.0)
    idx16 = np.zeros((N_CORES, S), dtype=np.int16)
    idx16.reshape(-1)[core * S + slot] = tbl_idx

    # wrap idxs: slot j -> partition j%16, col j//16; replicate to 128 partitions
    idx_w = idx16.reshape(N_CORES, S // 16, 16).transpose(0, 2, 1)
    idx_w = np.ascontiguousarray(idx_w)
    idx_w = np.tile(idx_w, (1, 8, 1))                  # [cores, 128, S/16]

    # gather tables: x rows pre-scaled by dinv, K-major within each rank half
    xp = np.zeros((N_PAD, IN_CH), dtype=np.float32)
    xp[:N_RAW] = x
    xp *= dinv[:, None]
    xp = xp.astype(NPBF16)
    xr = xp.reshape(N_CORES, SHARD, IN_CH)
    xA = np.ascontiguousarray(
        xr[:, :HALF_A].reshape(N_CORES, NBLK_A, 128, IN_CH)
        .transpose(0, 2, 1, 3).reshape(N_CORES * HALF_A, IN_CH))
    xB = np.ascontiguousarray(
        xr[:, HALF_A:].reshape(N_CORES, NBLK_B, 128, IN_CH)
        .transpose(0, 2, 1, 3).reshape(N_CORES * HALF_B, IN_CH))

    # per-core dinv columns: dinv_col[core][p, b] = dinv[core*SHARD + b*128 + p]
    dinv_col = np.ascontiguousarray(
        dinv.reshape(N_CORES, NBLK, 128).transpose(0, 2, 1))  # [cores, 128, NBLK]

    kAl = [int(v) for v in kA]
    kBl = [int(v) for v in kB]
    return dict(kA=kAl, kB=kBl, baseA=[int(v) for v in baseA],
                baseB=[int(v) for v in baseB], gArange=gArange, gBrange=gBrange,
                groups=groups, C=C, sel=sel, idx_w=idx_w, xA=xA, xB=xB,
                dinv_col=dinv_col)


# ----------------------------------------------------------- device program

def _build(prep):
    kA, kB = prep["kA"], prep["kB"]
    baseA, baseB = prep["baseA"], prep["baseB"]
    gArange, gBrange = prep["gArange"], prep["gBrange"]
    groups = prep["groups"]
    C = prep["C"]
    S = C * 128
    GCH = max(max(n for _, n in gArange), max(n for _, n in gBrange))

    nc = bacc.Bacc("TRN2", target_bir_lowering=False, num_devices=N_CORES,
                   num_swdge_queues=4)

    xA_d = nc.dram_tensor("xA", [N_CORES * HALF_A, IN_CH], BF16, kind="ExternalInput")
    xB_d = nc.dram_tensor("xB", [N_CORES * HALF_B, IN_CH], BF16, kind="ExternalInput")
    sel_d = nc.dram_tensor("sel", [128, C * 128], FP8, kind="ExternalInput")
    idx_d = nc.dram_tensor("idx", [128, S // 16], I16, kind="ExternalInput")
    dinv_d = nc.dram_tensor("dinv", [128, NBLK], FP32, kind="ExternalInput")
    w1_d = nc.dram_tensor("w1", [IN_CH, HID], BF16, kind="ExternalInput")
    w2_d = nc.dram_tensor("w2", [HID, OUT_CH], BF16, kind="ExternalInput")
    b1_d = nc.dram_tensor("b1", [1, HID], BF16, kind="ExternalInput")
    b2_d = nc.dram_tensor("b2", [1, OUT_CH], BF16, kind="ExternalInput")
    ident_d = nc.dram_tensor("ident", [128, 128], BF16, kind="ExternalInput")
    ones_d = nc.dram_tensor("ones", [1, 128], BF16, kind="ExternalInput")
    y_d = nc.dram_tensor("y", [SHARD, OUT_CH], FP32, kind="ExternalOutput")

    bncA = nc.dram_tensor("bncA", [128, NBLK_A * HID], BF16)
    bncB = nc.dram_tensor("bncB", [128, NBLK_B * HID], BF16)
    tA = nc.dram_tensor("tA", [N_CORES * HALF_A, HID], BF16, addr_space="Shared")
    tB = nc.dram_tensor("tB", [N_CORES * HALF_B, HID], BF16, addr_space="Shared")

    RG = [list(range(N_CORES))]
    ACT = mybir.ActivationFunctionType

    with TileContext(nc) as tc:
        nc.gpsimd.load_library(mlp)
        import contextlib
        st = contextlib.ExitStack()
        with st:
            consts = st.enter_context(tc.tile_pool(name="consts", bufs=1))
            fpool = st.enter_context(tc.tile_pool(name="fpool", bufs=4))
            ftpool = st.enter_context(tc.tile_pool(name="ftpool", bufs=4))
            opool = st.enter_context(tc.tile_pool(name="opool", bufs=4))
            stash = st.enter_context(tc.tile_pool(name="stash", bufs=NBLK))
            aggps = st.enter_context(tc.tile_pool(name="aggps", bufs=4, space="PSUM"))
            tps = st.enter_context(tc.tile_pool(name="tps", bufs=2, space="PSUM"))
            mmps = st.enter_context(tc.tile_pool(name="mmps", bufs=2, space="PSUM"))

            # ---- constants / resident tensors
            idx_t = consts.tile([128, S // 16], I16)
            nc.sync.dma_start(out=idx_t[:], in_=idx_d[:])
            sel_t = consts.tile([128, C, 128], FP8)
            NSEL = 4
            selsz = [(C // NSEL + (1 if i < C % NSEL else 0)) for i in range(NSEL)]
            off = 0
            for i, sz in enumerate(selsz):
                nc.sync.dma_start(out=sel_t[:, off:off + sz, :],
                                  in_=sel_d[:, off * 128:(off + sz) * 128])
                off += sz
            dinv_t = consts.tile([128, NBLK], FP32)
            nc.sync.dma_start(out=dinv_t[:], in_=dinv_d[:])
            w1_t = consts.tile([128, 2, HID], BF16)
            nc.sync.dma_start(out=w1_t[:], in_=w1_d.rearrange("(c k) m -> k c m", k=128))
            w2_t = consts.tile([128, OUT_CH], BF16)
            nc.sync.dma_start(out=w2_t[:], in_=w2_d[:])
            b1_t = consts.tile([1, HID], BF16)
            nc.sync.dma_start(out=b1_t[:], in_=b1_d[:])
            b2_t = consts.tile([1, OUT_CH], BF16)
            nc.sync.dma_start(out=b2_t[:], in_=b2_d[:])
            ones_t = consts.tile([1, 128], BF16)
            nc.sync.dma_start(out=ones_t[:], in_=ones_d[:])
            ident_t = consts.tile([128, 128], BF16)
            nc.sync.dma_start(out=ident_t[:], in_=ident_d[:])
            h2_t = consts.tile([128, NBLK, HID], BF16)

            qctr = [0]

            def gather(pool, tbl, tbl_ch, c0, n, layer_tag):
                """dma_gather of chunks [c0, c0+n) on a rotating queue."""
                assert n * 128 <= 2048, n
                q = qctr[0] % 4
                qctr[0] += 1
                g = pool.tile([128, GCH, tbl_ch], BF16, tag="g",
                              name=f"g{layer_tag}_{c0}")
                nc.gpsimd.dma_gather(
                    g[:, :n, :], tbl[:], idx_t[:, c0 * 8:(c0 + n) * 8],
                    n * 128, n * 128, tbl_ch, queue_num=q,
                    single_packet=(n * 128 <= 1024))
                return g

            def transform(b, aggf, Wt, nW, bias_t, out_cb):
                mp = mmps.tile([128, 128], FP32, tag="mmps", name=f"mm{id(aggf)}_{b}")
                for kc in range(nW):
                    tp = tps.tile([128, 128], BF16, tag="tp", name=f"tp{id(aggf)}_{b}_{kc}")
                    nc.tensor.transpose(tp[:], aggf[:, kc * 128:(kc + 1) * 128],
                                        ident_t[:])
                    ft = ftpool.tile([128, 128], BF16, tag="fT", name=f"fT{id(aggf)}_{b}_{kc}")
                    nc.scalar.activation(ft[:], tp[:], ACT.Copy)
                    nc.tensor.matmul(mp[:], ft[:], Wt(kc), start=(kc == 0), stop=False)
                nc.tensor.matmul(mp[:], ones_t[:], bias_t[:], start=False, stop=True)
                out_cb(b, mp)

            SKIP_L2 = bool(int(os.environ.get("GCN_SKIP_L2", "0")))
            SKIP_AG = bool(int(os.environ.get("GCN_SKIP_AG", "0")))

            # ================= layer 1: single phase, A/B interleaved =======
            def l1_out(b, mp):
                # h2 table row = relu(z) * dinv[dst]  (== relu(z*dinv), dinv>0)
                nc.scalar.activation(h2_t[:, b, :], mp[:], ACT.Relu,
                                     scale=dinv_t[:, b:b + 1])
                if SKIP_AG:
                    return
                if b == NBLK_A - 1:
                    nc.sync.dma_start(out=bncA[:], in_=h2_t[:, :NBLK_A, :])
                    nc.gpsimd.collective_compute(
                        "AllGather", mybir.AluOpType.bypass, replica_groups=RG,
                        ins=[bncA[:]], outs=[tA[:]])
                elif b == NBLK - 1:
                    nc.sync.dma_start(out=bncB[:], in_=h2_t[:, NBLK_A:, :])
                    nc.gpsimd.collective_compute(
                        "AllGather", mybir.AluOpType.bypass, replica_groups=RG,
                        ins=[bncB[:]], outs=[tB[:]])

            with tc.tile_pool(name="gpool1", bufs=8) as gpool1:
                for gi, blks in enumerate(groups):
                    a0, na = gArange[gi]
                    b0, nb = gBrange[gi]
                    gA = gather(gpool1, xA_d, IN_CH, a0, na, "1a")
                    gB = gather(gpool1, xB_d, IN_CH, b0, nb, "1b")
                    for b in blks:
                        ps = aggps.tile([128, IN_CH], FP32, tag="aggps",
                                        name=f"ps1_{b}")
                        for j in range(kA[b]):
                            cg = baseA[b] + j
                            nc.tensor.matmul(ps[:], sel_t[:, cg, :],
                                             gA[:, cg - a0, :], start=(j == 0),
                                             stop=False)
                        for j in range(kB[b]):
                            cg = baseB[b] + j
                            nc.tensor.matmul(ps[:], sel_t[:, cg, :],
                                             gB[:, cg - b0, :], start=False,
                                             stop=(j == kB[b] - 1))
                        aggf = fpool.tile([128, IN_CH], BF16, tag="aggf",
                                          name=f"aggf1_{b}")
                        nc.vector.tensor_scalar_mul(aggf[:], ps[:],
                                                    dinv_t[:, b:b + 1])
                        transform(b, aggf, lambda kc: w1_t[:, kc, :], 2, b1_t,
                                  l1_out)

            # ================= layer 2: phase A (tA), then phase B (tB) =====
            def l2_out(b, mp):
                o = opool.tile([128, OUT_CH], FP32, tag="o", name=f"y{b}")
                nc.scalar.activation(o[:], mp[:], ACT.Copy)
                nc.sync.dma_start(out=y_d[b * 128:(b + 1) * 128, :], in_=o[:])

            if SKIP_L2:
                for b in range(NBLK):
                    o = opool.tile([128, OUT_CH], FP32, tag="o", name=f"yd{b}")
                    nc.scalar.activation(o[:], h2_t[:, b, :], ACT.Copy)
                    nc.sync.dma_start(out=y_d[b * 128:(b + 1) * 128, :], in_=o[:])
            aggA = {}
            with tc.tile_pool(name="gpool2", bufs=8) as gpool2:
                if SKIP_L2:
                    groups_eff = []
                else:
                    groups_eff = groups
                for gi, blks in enumerate(groups_eff):
                    a0, na = gArange[gi]
                    gA = gather(gpool2, tA, HID, a0, na, "2a")
                    for b in blks:
                        ps = aggps.tile([128, HID], FP32, tag="aggps",
                                        name=f"ps2a_{b}")
                        for j in range(kA[b]):
                            cg = baseA[b] + j
                            nc.tensor.matmul(ps[:], sel_t[:, cg, :],
                                             gA[:, cg - a0, :], start=(j == 0),
                                             stop=(j == kA[b] - 1))
                        a = stash.tile([128, HID], BF16, tag="aggA",
                                       name=f"aggA_{b}")
                        aggA[b] = a
                        nc.scalar.activation(a[:], ps[:], ACT.Copy)
                for gi, blks in enumerate(groups_eff):
                    b0, nb = gBrange[gi]
                    gB = gather(gpool2, tB, HID, b0, nb, "2b")
                    for b in blks:
                        ps = aggps.tile([128, HID], FP32, tag="aggps",
                                        name=f"ps2b_{b}")
                        nc.tensor.matmul(ps[:], ident_t[:], aggA[b][:],
                                         start=True, stop=False)
                        for j in range(kB[b]):
                            cg = baseB[b] + j
                            nc.tensor.matmul(ps[:], sel_t[:, cg, :],
                                             gB[:, cg - b0, :], start=False,
                                             stop=(j == kB[b] - 1))
                        aggf = fpool.tile([128, IN_CH], BF16, tag="aggf",
                                          name=f"aggf2_{b}")
                        nc.vector.tensor_scalar_mul(aggf[:, :HID], ps[:],
                                                    dinv_t[:, b:b + 1])
                        transform(b, aggf, lambda kc: w2_t[:], 1, b2_t,
                                  l2_out)

    nc.compile()
    return nc


# ------------------------------------------------------------------- kernel

def kernel(x, edge_index, W1, b1, W2, b2):
    global last_exec_time_ns, last_results
    x = np.asarray(x, dtype=np.float32)
    prep = _prep(x, np.asarray(edge_index))
    nc = _build(prep)

    w1b = np.asarray(W1, dtype=np.float32).astype(NPBF16)
    w2b = np.asarray(W2, dtype=np.float32).astype(NPBF16)
    b1b = np.asarray(b1, dtype=np.float32).reshape(1, -1).astype(NPBF16)
    b2b = np.asarray(b2, dtype=np.float32).reshape(1, -1).astype(NPBF16)
    ident = np.zeros((128, 128), dtype=NPBF16)
    ident[np.arange(128), np.arange(128)] = 1.0

    in_maps = []
    for c in range(N_CORES):
        in_maps.append({
            "xA": prep["xA"], "xB": prep["xB"],
            "sel": prep["sel"][c].reshape(128, -1), "idx": prep["idx_w"][c],
            "dinv": prep["dinv_col"][c],
            "w1": w1b, "w2": w2b, "b1": b1b, "b2": b2b, "ident": ident,
            "ones": np.ones((1, 128), dtype=NPBF16),
        })

    trace = bool(int(os.environ.get("GCN_TRACE", "0")))
    if trace:
        try:
            import ntff_shim
            ntff_shim.install()
        except Exception:
            trace = False
    res = run_bass_kernel_spmd(nc, in_maps, list(range(N_CORES)), trace=trace)
    last_exec_time_ns = res.exec_time_ns
    last_results = res

    y = np.concatenate([np.asarray(res.results[c]["y"]) for c in range(N_CORES)], axis=0)
    return np.ascontiguousarray(y[:N_RAW]).astype(np.float32)
